# revision 4
# baseline (speedup 1.0000x reference)
"""AttenNetVLAD Trainium2 kernel (8-core data parallel).

Reference computation per batch n (C=512 channels, P=1600 pixels, K=64 clusters):
  hmp   = relu(attn_w . relu(x) + attn_b)                    # [P]
  xn    = x / max(||x||_c, eps)                              # [C,P]
  sa    = softmax_k(conv_w @ xn)                             # [K,P]
  w     = sa * hmp
  vlad  = l2norm_glob(l2norm_c(w @ xn^T - (w.1) * centroids))

Strategy: batch data-parallel over 8 cores (6 batches each). Per batch:
  - SWDGE DMA loads x fp32->bf16 into [c,p] tiles (cast rides the DMA).
  - xbar DMA-transpose (bf16) produces xT [p,c] tiles (mapping: src col j ->
    partition j%128, chunk j//128 -- verified on HW).
  - norm2 via ACT Square+accum_out from xT; hmp via DVE scalar_tensor_tensor
    (relu * attn_w)+accum_out from xT.
  - logitsT [p,k] on PE: x [c,p] slices stationary, conv_w^T moving (bf16,
    fp32 PSUM accumulate).
  - ACT Exp with per-partition scale=inv_norm and accum_out=sumexp.
  - w~ = exp * (hmp*inv_norm/sumexp) folded as one per-partition scale; the
    extra inv_norm folds xn's normalization into w~ so term1 can use raw x.
  - term1 [k,c] + wsum on PE contracting p (w~ stationary, xT moving).
  - vlad assembly + both l2 norms as per-partition scales; sign of
    (wsum*cen - term1) fixed by negating the final scale.
"""

import numpy as np
from contextlib import ExitStack

import concourse.bass as bass
import concourse.bacc as bacc
import concourse.bass_isa as bass_isa
import concourse.tile as tile
from concourse import mybir
from concourse.bass_utils import run_bass_kernel_spmd

F32 = mybir.dt.float32
BF16 = mybir.dt.bfloat16
ALU = mybir.AluOpType
ACTF = mybir.ActivationFunctionType

N_CORES = 8
NB = 6            # batches per core
C = 512
P = 1600
K = 64
CB = 4            # channel blocks of 128
PCH = 13          # p chunks of 128
PPAD = PCH * 128  # 1664

_CACHE = {}


def _bcast_ap(handle_ap, parts, free_ap):
    return bass.AP(tensor=handle_ap.tensor, offset=handle_ap.offset,
                   ap=[[0, parts]] + free_ap)


def _build():
    nc = bacc.Bacc("TRN2", target_bir_lowering=False, debug=False,
                   num_devices=N_CORES)
    x_in = nc.declare_dram_parameter("x", [NB, C, P], F32, isOutput=False)
    cw_in = nc.declare_dram_parameter("conv_w", [K, C], F32, isOutput=False)
    aw_in = nc.declare_dram_parameter("attn_w", [1, C], F32, isOutput=False)
    ab_in = nc.declare_dram_parameter("attn_b", [1], F32, isOutput=False)
    cen_in = nc.declare_dram_parameter("centroids", [K, C], F32, isOutput=False)
    out_p = nc.declare_dram_parameter("out", [NB, K * C], F32, isOutput=True)
    out_v = out_p[:, :].rearrange("n (k c) -> n k c", k=K)

    with tile.TileContext(nc) as tc, ExitStack() as ctx:
        const = ctx.enter_context(tc.tile_pool(name="const", bufs=1))
        big = ctx.enter_context(tc.tile_pool(name="big", bufs=2))
        med = ctx.enter_context(tc.tile_pool(name="med", bufs=2))
        ps_log = ctx.enter_context(tc.tile_pool(name="ps_log", bufs=3, space="PSUM"))
        ps_t1 = ctx.enter_context(tc.tile_pool(name="ps_t1", bufs=2, space="PSUM"))
        ps_ws = ctx.enter_context(tc.tile_pool(name="ps_ws", bufs=2, space="PSUM"))

        # ---- constants ----
        cw_f = const.tile([K, C], F32)
        nc.sync.dma_start(out=cw_f, in_=cw_in[:, :])
        cw_b = const.tile([K, C], BF16)
        nc.vector.tensor_copy(cw_b, cw_f)
        cwT = const.tile([128, CB, K], BF16)   # cwT[cc, cb, k] = conv_w[k, cb*128+cc]
        for cb in range(CB):
            nc.sync.dma_start_transpose(out=cwT[:, cb, :],
                                        in_=cw_b[:, cb * 128:(cb + 1) * 128])
        awB = const.tile([128, CB, 128], F32)  # attn_w broadcast to all partitions
        nc.gpsimd.dma_start(out=awB, in_=_bcast_ap(aw_in[:, :], 128, [[128, CB], [1, 128]]))
        bB = const.tile([128, 1], F32)
        nc.gpsimd.dma_start(out=bB, in_=_bcast_ap(ab_in[:], 128, [[1, 1]]))
        cen = const.tile([K, C], F32)
        nc.sync.dma_start(out=cen, in_=cen_in[:, :])

        for b in range(NB):
            # ---- load (fp32->bf16 cast in DMA) + zero pad cols ----
            xb = big.tile([128, CB, PPAD], BF16, tag="xb")
            nc.vector.memset(xb[:, :, P:PPAD], 0.0)
            nc.gpsimd.dma_start(
                out=xb[:, :, 0:P],
                in_=x_in[b].rearrange("(cb cc) p -> cc cb p", cc=128))

            # ---- xbar transpose: xt[pp, cb, ch, cc] = x[cb*128+cc, ch*128+pp] ----
            xt = big.tile([128, CB, PCH, 128], BF16, tag="xt")
            for cb in range(CB):
                nc.sync.dma_start_transpose(out=xt[:, cb], in_=xb[:, cb, :])

            # ---- norm2 (ACT) and hmp (DVE) from xT ----
            norm2 = med.tile([128, PCH], F32, tag="n2")
            hmp0 = med.tile([128, PCH], F32, tag="h0")
            junkA = med.tile([128, CB, 128], BF16, tag="jA")
            junkD = med.tile([128, CB, 128], BF16, tag="jD")
            for ch in range(PCH):
                nc.scalar.activation(out=junkA, in_=xt[:, :, ch, :], func=ACTF.Square,
                                     accum_out=norm2[:, ch:ch + 1])
                nc.vector.scalar_tensor_tensor(
                    out=junkD, in0=xt[:, :, ch, :], scalar=0.0, in1=awB,
                    op0=ALU.max, op1=ALU.mult, accum_out=hmp0[:, ch:ch + 1])

            nrm = med.tile([128, PCH], F32, tag="nrm")
            nc.scalar.activation(out=nrm, in_=norm2, func=ACTF.Sqrt)
            nc.vector.tensor_scalar_max(nrm, nrm, 1e-12)
            invn = med.tile([128, PCH], F32, tag="invn")
            nc.vector.reciprocal(invn, nrm)
            # bf16 norm column: wsum's matmul rhs, cancelling the inv_norm
            # factor folded into w~ (wsum = sum_p sa*hmp, no inv_norm).
            nrmb = med.tile([128, PCH], BF16, tag="nrmb")
            nc.vector.tensor_copy(nrmb, nrm)

            # ---- logitsT chunks on PE; exp+sumexp on ACT ----
            expw = big.tile([128, PCH, K], F32, tag="expw")
            sume = med.tile([128, PCH], F32, tag="sume")
            for ch in range(PCH):
                lps = ps_log.tile([128, K], F32, tag="lps")
                for cb in range(CB):
                    nc.tensor.matmul(lps,
                                     xb[:, cb, ch * 128:(ch + 1) * 128],
                                     cwT[:, cb, :],
                                     start=(cb == 0), stop=(cb == CB - 1))
                nc.scalar.activation(out=expw[:, ch, :], in_=lps, func=ACTF.Exp,
                                     scale=invn[:, ch:ch + 1],
                                     accum_out=sume[:, ch:ch + 1])

            # ---- srow = hmp * invn / sumexp ; w~ = expw * srow (bf16) ----
            hmp = med.tile([128, PCH], F32, tag="hmp")
            nc.vector.tensor_scalar(out=hmp, in0=hmp0, scalar1=bB, scalar2=0.0,
                                    op0=ALU.add, op1=ALU.max)
            rcs = med.tile([128, PCH], F32, tag="rcs")
            nc.vector.reciprocal(rcs, sume)
            srow = med.tile([128, PCH], F32, tag="srow")
            nc.vector.tensor_mul(srow, hmp, invn)
            nc.vector.tensor_mul(srow, srow, rcs)
            wt = med.tile([128, PCH, K], BF16, tag="wt")
            for ch in range(PCH):
                nc.vector.tensor_scalar_mul(wt[:, ch, :], expw[:, ch, :],
                                            srow[:, ch:ch + 1])

            # ---- term1 [k,c] and wsum [k] on PE (contract p) ----
            t1 = ps_t1.tile([K, C], F32, tag="t1")
            ws = ps_ws.tile([K, 1], F32, tag="ws")
            for ch in range(PCH):
                nc.tensor.matmul(t1, wt[:, ch, :], xt[:, :, ch, :],
                                 start=(ch == 0), stop=(ch == PCH - 1))
                nc.tensor.matmul(ws, wt[:, ch, :], nrmb[:, ch:ch + 1],
                                 start=(ch == 0), stop=(ch == PCH - 1))

            # ---- vlad assembly + normalization ----
            vneg = med.tile([K, C], F32, tag="vneg")   # wsum*cen - term1 = -vlad
            nc.vector.scalar_tensor_tensor(out=vneg, in0=cen, scalar=ws, in1=t1,
                                           op0=ALU.mult, op1=ALU.subtract)
            junkK = med.tile([K, C], BF16, tag="jK")
            ssq = med.tile([K, 1], F32, tag="ssq")
            nc.scalar.activation(out=junkK, in_=vneg, func=ACTF.Square,
                                 accum_out=ssq)
            sn = med.tile([K, 1], F32, tag="sn")
            nc.scalar.activation(out=sn, in_=ssq, func=ACTF.Sqrt)
            nc.vector.tensor_scalar_max(sn, sn, 1e-12)
            inv1 = med.tile([K, 1], F32, tag="inv1")
            nc.vector.reciprocal(inv1, sn)
            # global norm: sum_k of per-row normalized squares
            rn2 = med.tile([K, 1], F32, tag="rn2")
            nc.vector.tensor_mul(rn2, ssq, inv1)
            nc.vector.tensor_mul(rn2, rn2, inv1)
            g2 = med.tile([K, 1], F32, tag="g2")
            nc.gpsimd.partition_all_reduce(g2, rn2, channels=K,
                                           reduce_op=bass_isa.ReduceOp.add)
            gn = med.tile([K, 1], F32, tag="gn")
            nc.scalar.activation(out=gn, in_=g2, func=ACTF.Sqrt)
            nc.vector.tensor_scalar_max(gn, gn, 1e-12)
            invg = med.tile([K, 1], F32, tag="invg")
            nc.vector.reciprocal(invg, gn)
            sfin = med.tile([K, 1], F32, tag="sfin")   # -(inv1*invg) fixes sign
            nc.vector.scalar_tensor_tensor(out=sfin, in0=inv1, scalar=-1.0,
                                           in1=invg, op0=ALU.mult, op1=ALU.mult)
            outb = med.tile([K, C], F32, tag="outb")
            nc.scalar.activation(out=outb, in_=vneg, func=ACTF.Copy, scale=sfin)
            nc.sync.dma_start(out=out_v[b], in_=outb)

    nc.finalize()
    return nc


def kernel(x, conv_w, attn_w, attn_b, centroids):
    x = np.ascontiguousarray(np.asarray(x, dtype=np.float32)).reshape(48, C, P)
    conv_w = np.ascontiguousarray(np.asarray(conv_w, dtype=np.float32))
    attn_w = np.ascontiguousarray(np.asarray(attn_w, dtype=np.float32)).reshape(1, C)
    attn_b = np.ascontiguousarray(np.asarray(attn_b, dtype=np.float32)).reshape(1)
    centroids = np.ascontiguousarray(np.asarray(centroids, dtype=np.float32))

    if "nc" not in _CACHE:
        _CACHE["nc"] = _build()
    nc = _CACHE["nc"]

    in_maps = []
    for i in range(N_CORES):
        in_maps.append({
            "x": x[i * NB:(i + 1) * NB],
            "conv_w": conv_w,
            "attn_w": attn_w,
            "attn_b": attn_b,
            "centroids": centroids,
        })
    res = run_bass_kernel_spmd(nc, in_maps, list(range(N_CORES)))
    out = np.concatenate([res.results[i]["out"] for i in range(N_CORES)], axis=0)
    return out.astype(np.float32)
